# revision 9
# baseline (speedup 1.0000x reference)
"""Multi-head dot-product attention on 8 TRN2 NeuronCores.

Problem: B=4, S=2048, D=1024, H=16, DH=64 (fp32 reference).

Sharding: 8 shards = 4 batches x 2 head-halves. Each core computes, for one
batch b and 8 heads, the QKV projections, attention, and its partial output
projection. The host sums the two half-head partials per batch (the Wo
contraction all-reduce) and adds bo.

Per-core kernel layout (all matmul contraction dims on SBUF partitions):
  - XqT/XkvT: x loaded transposed, [D(128-tiles), S] fp32; projections run in
    float32r (full-rate PE) so no input casts are needed.
  - QT/KT: [128 = head-pair (2x64 dh), S] fp16 - produced directly transposed
    by using W as lhsT. Head pairs stacked so that the K=64 scores matmuls for
    the two heads row-pack onto the PE array (tile_position rows 0-63/64-127).
  - scoresT: [k-tile 128, q 1024] PSUM; exp on ACT (scale=1/8 folded in,
    no max-subtraction: scores ~ N(0,1), |s| < ~7, exp is safe in fp32/fp16).
  - softmax denominators: DVE accumulates expT k-tiles into an fp16 partial-sum
    tile; a ones[128,64] matmul reduces partitions AND broadcasts, giving
    per-head denominator rows aligned with xT; fast reciprocal on DVE.
  - PV: xT[dh, q] accumulated over k-tiles, two heads col-packed
    (tile_position cols 0-63/64-127) -> xT stacked [128, q] ready as lhsT for
    the Wo projection. Normalization fused into the PSUM->SBUF evacuation.
  - out projection: out[q,d] accumulated over 4 head-pairs, DMA'd to DRAM
    straight from PSUM.
"""

import os

import numpy as np

import concourse.bass as bass
from concourse import bacc
import concourse.mybir as mybir
import concourse.tile as tile
from concourse.bass_utils import run_bass_kernel_spmd

B, S, D, H, DH = 4, 2048, 1024, 16, 64
P = 128
HC = H // 2          # heads per core = 8
PAIRS = HC // 2      # head pairs per core = 4
DT = D // P          # projection contraction tiles = 8
NKT = S // P         # key tiles = 16
QC = 1024            # q chunk per psum tile
NQC = S // QC        # 2
NSUB = QC // 512     # matmul sub-chunks per psum tile
HDH = HC * DH        # per-core Wo contraction = 512

F32 = mybir.dt.float32
F16 = mybir.dt.float16
EXP = mybir.ActivationFunctionType.Exp


def _emit(nc):
    xq = nc.dram_tensor("xq", [S, D], F32, kind="ExternalInput")
    xkv = nc.dram_tensor("xkv", [S, D], F32, kind="ExternalInput")
    wq = nc.dram_tensor("wq", [D, HDH], F32, kind="ExternalInput")
    wk = nc.dram_tensor("wk", [D, HDH], F32, kind="ExternalInput")
    wv = nc.dram_tensor("wv", [D, HDH], F32, kind="ExternalInput")
    bq = nc.dram_tensor("bq", [HDH], F32, kind="ExternalInput")
    bk = nc.dram_tensor("bk", [HDH], F32, kind="ExternalInput")
    bv = nc.dram_tensor("bv", [HDH], F32, kind="ExternalInput")
    wo = nc.dram_tensor("wo", [HDH, D], F32, kind="ExternalInput")
    out = nc.dram_tensor("out", [S, D], F32, kind="ExternalOutput")

    with tile.TileContext(nc) as tc:
        with tc.tile_pool(name="persist", bufs=1) as pers:
            # persistent SBUF tensors
            qt_sb = [pers.tile([P, S], F16, tag=f"qt{t}", name=f"qt{t}") for t in range(PAIRS)]
            kt_sb = [pers.tile([P, S], F16, tag=f"kt{t}", name=f"kt{t}") for t in range(PAIRS)]
            v_sb = [pers.tile([P, HDH], F16, tag=f"v{st}", name=f"v{st}") for st in range(NKT)]
            wo_sb = [pers.tile([P, D], F16, tag=f"wo{t}", name=f"wo{t}") for t in range(PAIRS)]
            ones_mm = pers.tile([1, 512], F16, tag="ones_mm")
            ones_red = pers.tile([P, 64], F16, tag="ones_red")
            bq_sb = pers.tile([1, HDH], F16, tag="bq")
            bk_sb = pers.tile([1, HDH], F16, tag="bk")
            bv_sb = pers.tile([1, HDH], F16, tag="bv")

            nc.vector.memset(ones_mm, 1.0)
            nc.vector.memset(ones_red, 1.0)
            nc.gpsimd.dma_start(out=bq_sb, in_=bq[None, :])
            nc.gpsimd.dma_start(out=bk_sb, in_=bk[None, :])
            nc.gpsimd.dma_start(out=bv_sb, in_=bv[None, :])

            # ---------------- Phase 1: projections ----------------
            with (
                tc.tile_pool(name="xt", bufs=9) as xt_pool,
                tc.tile_pool(name="stg", bufs=3) as stg_pool,
                tc.tile_pool(name="w", bufs=16) as w_pool,
                tc.tile_pool(name="pproj", bufs=4, space="PSUM") as pj,
            ):
                # Wo load (SWDGE cast-DMA fp32 -> fp16)
                for t in range(PAIRS):
                    nc.gpsimd.dma_start(out=wo_sb[t], in_=wo[t * P : (t + 1) * P, :])

                def load_xT(x_dram):
                    # HWDGE gather of the transposed slab (fp32), then GPSIMD
                    # cast-copy to fp16 (keeps DVE free for softmax sums).
                    tiles = []
                    for d in range(DT):
                        stg = stg_pool.tile([P, S], F32, tag="stg")
                        nc.sync.dma_start(
                            out=stg,
                            in_=x_dram[:, d * P : (d + 1) * P].rearrange("s d -> d s"),
                        )
                        t_ = xt_pool.tile([P, S], F16, tag="xt")
                        nc.gpsimd.tensor_copy(out=t_, in_=stg)
                        tiles.append(t_)
                    return tiles

                def load_w(w_dram):
                    tiles = []
                    for d in range(DT):
                        t_ = w_pool.tile([P, HDH], F16, tag="w")
                        nc.gpsimd.dma_start(out=t_, in_=w_dram[d * P : (d + 1) * P, :])
                        tiles.append(t_)
                    return tiles

                def proj_T(x_tiles, w_tiles, b_sb, out_tiles):
                    # out_tiles[pair][128 = pair-dh, S] = W.T @ X.T + b
                    for t in range(PAIRS):
                        for c in range(S // 512):
                            ps = pj.tile([P, 512], F32, tag="pj")
                            for d in range(DT):
                                nc.tensor.matmul(
                                    ps,
                                    lhsT=w_tiles[d][:, t * P : (t + 1) * P],
                                    rhs=x_tiles[d][:, c * 512 : (c + 1) * 512],
                                    start=(d == 0),
                                    stop=False,
                                )
                            nc.tensor.matmul(
                                ps,
                                lhsT=b_sb[:, t * P : (t + 1) * P],
                                rhs=ones_mm,
                                start=False,
                                stop=True,
                            )
                            nc.vector.tensor_copy(
                                out=out_tiles[t][:, c * 512 : (c + 1) * 512], in_=ps
                            )

                xq_t = load_xT(xq)
                wq_t = load_w(wq)
                wk_t = load_w(wk)
                proj_T(xq_t, wq_t, bq_sb, qt_sb)

                xkv_t = load_xT(xkv)
                proj_T(xkv_t, wk_t, bk_sb, kt_sb)

                wv_t = load_w(wv)
                # V natural layout: [s-tile 128, (h dh) 512] = X @ Wv + bv
                for st in range(NKT):
                    ps = pj.tile([P, 512], F32, tag="pj")
                    for d in range(DT):
                        nc.tensor.matmul(
                            ps,
                            lhsT=xkv_t[d][:, st * P : (st + 1) * P],
                            rhs=wv_t[d],
                            start=(d == 0),
                            stop=False,
                        )
                    nc.tensor.matmul(
                        ps,
                        lhsT=ones_mm[:, :P],
                        rhs=bv_sb,
                        start=False,
                        stop=True,
                    )
                    nc.vector.tensor_copy(out=v_sb[st], in_=ps)

            # ---------------- Phase 2: attention + out projection ----------------
            with (
                tc.tile_pool(name="psc", bufs=2, space="PSUM") as psc,
                tc.tile_pool(name="pxt", bufs=1, space="PSUM") as pxt,
                tc.tile_pool(name="pout", bufs=2, space="PSUM") as pout,
                tc.tile_pool(name="et", bufs=4) as et_pool,
                tc.tile_pool(name="accp", bufs=4) as acc_pool,
                tc.tile_pool(name="rec", bufs=2) as rec_pool,
                tc.tile_pool(name="xtsb", bufs=8) as xtsb_pool,
            ):
                for qc in range(NQC):
                    xts = []
                    for pr in range(PAIRS):
                        h0, h1 = 2 * pr, 2 * pr + 1
                        acc0 = acc_pool.tile([P, QC], F16, tag="acc")
                        acc1 = acc_pool.tile([P, QC], F16, tag="acc")
                        xt_ps = pxt.tile([P, QC], F32, tag="xt")
                        for kt in range(NKT):
                            ps_a = psc.tile([P, QC], F32, tag="sc")
                            ps_b = psc.tile([P, QC], F32, tag="sc")
                            ksl = slice(kt * P, (kt + 1) * P)
                            for sub in range(NSUB):
                                sl = slice(sub * 512, (sub + 1) * 512)
                                qsl = slice(
                                    qc * QC + sub * 512, qc * QC + (sub + 1) * 512
                                )
                                # scoresT[k, q] for the two heads, row-packed
                                nc.tensor.matmul(
                                    ps_a[:, sl],
                                    lhsT=kt_sb[pr][0:64, ksl],
                                    rhs=qt_sb[pr][0:64, qsl],
                                    start=True,
                                    stop=True,
                                    tile_position=(0, 0),
                                )
                                nc.tensor.matmul(
                                    ps_b[:, sl],
                                    lhsT=kt_sb[pr][64:128, ksl],
                                    rhs=qt_sb[pr][64:128, qsl],
                                    start=True,
                                    stop=True,
                                    tile_position=(64, 0),
                                )
                            et0 = et_pool.tile([P, QC], F16, tag="et")
                            et1 = et_pool.tile([P, QC], F16, tag="et")
                            nc.scalar.activation(out=et0, in_=ps_a, func=EXP, scale=0.125)
                            nc.scalar.activation(out=et1, in_=ps_b, func=EXP, scale=0.125)
                            # partial softmax denominators (fp16, 16 terms per lane)
                            if kt == 0:
                                nc.vector.tensor_copy(out=acc0, in_=et0)
                                nc.vector.tensor_copy(out=acc1, in_=et1)
                            else:
                                nc.vector.tensor_add(out=acc0, in0=acc0, in1=et0)
                                nc.vector.tensor_add(out=acc1, in0=acc1, in1=et1)
                            # xT[dh, q] += V[k-tile].T-slice @ expT, heads col-packed
                            for sub in range(NSUB):
                                sl = slice(sub * 512, (sub + 1) * 512)
                                nc.tensor.matmul(
                                    xt_ps[0:64, sl],
                                    lhsT=v_sb[kt][:, h0 * DH : (h0 + 1) * DH],
                                    rhs=et0[:, sl],
                                    start=(kt == 0),
                                    stop=(kt == NKT - 1),
                                    tile_position=(0, 0),
                                    skip_group_check=True,
                                )
                                nc.tensor.matmul(
                                    xt_ps[64:128, sl],
                                    lhsT=v_sb[kt][:, h1 * DH : (h1 + 1) * DH],
                                    rhs=et1[:, sl],
                                    start=(kt == 0),
                                    stop=(kt == NKT - 1),
                                    tile_position=(0, 64),
                                    skip_group_check=True,
                                )
                        # denominators: partition-reduce + broadcast in one matmul
                        bs = psc.tile([P, QC], F32, tag="sc")
                        for sub in range(NSUB):
                            sl = slice(sub * 512, (sub + 1) * 512)
                            nc.tensor.matmul(
                                bs[0:64, sl],
                                lhsT=ones_red,
                                rhs=acc0[:, sl],
                                start=True,
                                stop=True,
                                tile_position=(0, 0),
                                skip_group_check=True,
                            )
                            nc.tensor.matmul(
                                bs[64:128, sl],
                                lhsT=ones_red,
                                rhs=acc1[:, sl],
                                start=True,
                                stop=True,
                                tile_position=(0, 64),
                                skip_group_check=True,
                            )
                        rec = rec_pool.tile([P, QC], F32, tag="rec")
                        nc.vector.reciprocal_approx_fast(out=rec, in_=bs)
                        xt_sb = xtsb_pool.tile([P, QC], F16, tag="xtsb")
                        nc.vector.tensor_mul(out=xt_sb, in0=xt_ps, in1=rec)
                        xts.append(xt_sb)
                    # out projection for this q chunk
                    for qt_ in range(QC // P):
                        for dc in range(D // 512):
                            po = pout.tile([P, 512], F32, tag="po")
                            for pr in range(PAIRS):
                                nc.tensor.matmul(
                                    po,
                                    lhsT=xts[pr][:, qt_ * P : (qt_ + 1) * P],
                                    rhs=wo_sb[pr][:, dc * 512 : (dc + 1) * 512],
                                    start=(pr == 0),
                                    stop=(pr == PAIRS - 1),
                                )
                            osb = xtsb_pool.tile([P, 512], F32, tag="osb")
                            nc.vector.tensor_copy(out=osb, in_=po)
                            q0 = qc * QC + qt_ * P
                            nc.gpsimd.dma_start(
                                out=out[q0 : q0 + P, dc * 512 : (dc + 1) * 512],
                                in_=osb,
                            )
    return nc


_NC_CACHE = None
LAST_RESULTS = None


def _get_nc():
    global _NC_CACHE
    if _NC_CACHE is None:
        nc = bacc.Bacc(None, target_bir_lowering=False)
        _emit(nc)
        nc.compile()
        _NC_CACHE = nc
    return _NC_CACHE


def kernel(**inputs):
    global LAST_RESULTS
    inputs_q = np.ascontiguousarray(inputs["inputs_q"], np.float32)
    inputs_kv = np.ascontiguousarray(inputs["inputs_kv"], np.float32)
    Wq = np.asarray(inputs["Wq"], np.float32)
    Wk = np.asarray(inputs["Wk"], np.float32)
    Wv = np.asarray(inputs["Wv"], np.float32)
    bq = np.asarray(inputs["bq"], np.float32)
    bk = np.asarray(inputs["bk"], np.float32)
    bv = np.asarray(inputs["bv"], np.float32)
    Wo = np.asarray(inputs["Wo"], np.float32)
    bo = np.asarray(inputs["bo"], np.float32)

    nc = _get_nc()

    in_maps = []
    for core in range(8):
        b, g = core // 2, core % 2
        hsl = slice(g * HC, (g + 1) * HC)
        in_maps.append(
            {
                "xq": inputs_q[b],
                "xkv": inputs_kv[b],
                "wq": np.ascontiguousarray(Wq[:, hsl, :].reshape(D, HDH)),
                "wk": np.ascontiguousarray(Wk[:, hsl, :].reshape(D, HDH)),
                "wv": np.ascontiguousarray(Wv[:, hsl, :].reshape(D, HDH)),
                "bq": np.ascontiguousarray(bq[hsl].reshape(HDH)),
                "bk": np.ascontiguousarray(bk[hsl].reshape(HDH)),
                "bv": np.ascontiguousarray(bv[hsl].reshape(HDH)),
                "wo": np.ascontiguousarray(Wo[hsl].reshape(HDH, D)),
            }
        )

    res = run_bass_kernel_spmd(
        nc,
        in_maps,
        core_ids=list(range(8)),
        trace=bool(int(os.environ.get("KERNEL_TRACE", "0"))),
    )
    LAST_RESULTS = res

    out = np.empty((B, S, D), np.float32)
    for b in range(B):
        out[b] = res.results[2 * b]["out"] + res.results[2 * b + 1]["out"] + bo
    return out


# revision 10
# speedup vs baseline: 2.6491x; 2.6491x over previous
"""Multi-head dot-product attention on 8 TRN2 NeuronCores.

Problem: B=4, S=2048, D=1024, H=16, DH=64 (fp32 reference).

Sharding: 8 shards = 4 batches x 2 head-halves. Each core computes, for one
batch b and 8 heads, the QKV projections, attention, and its partial output
projection. The host sums the two half-head partials per batch (the Wo
contraction all-reduce) and adds bo.

Per-core kernel layout (all matmul contraction dims on SBUF partitions):
  - XqT/XkvT: x loaded transposed, [D(128-tiles), S] fp32; projections run in
    float32r (full-rate PE) so no input casts are needed.
  - QT/KT: [128 = head-pair (2x64 dh), S] fp16 - produced directly transposed
    by using W as lhsT. Head pairs stacked so that the K=64 scores matmuls for
    the two heads row-pack onto the PE array (tile_position rows 0-63/64-127).
  - scoresT: [k-tile 128, q 1024] PSUM; exp on ACT (scale=1/8 folded in,
    no max-subtraction: scores ~ N(0,1), |s| < ~7, exp is safe in fp32/fp16).
  - softmax denominators: DVE accumulates expT k-tiles into an fp16 partial-sum
    tile; a ones[128,64] matmul reduces partitions AND broadcasts, giving
    per-head denominator rows aligned with xT; fast reciprocal on DVE.
  - PV: xT[dh, q] accumulated over k-tiles, two heads col-packed
    (tile_position cols 0-63/64-127) -> xT stacked [128, q] ready as lhsT for
    the Wo projection. Normalization fused into the PSUM->SBUF evacuation.
  - out projection: out[q,d] accumulated over 4 head-pairs, DMA'd to DRAM
    straight from PSUM.
"""

import os

import numpy as np

import concourse.bass as bass
from concourse import bacc
import concourse.mybir as mybir
import concourse.tile as tile
from concourse.bass_utils import run_bass_kernel_spmd

B, S, D, H, DH = 4, 2048, 1024, 16, 64
P = 128
HC = H // 2          # heads per core = 8
PAIRS = HC // 2      # head pairs per core = 4
DT = D // P          # projection contraction tiles = 8
NKT = S // P         # key tiles = 16
QC = 1024            # q chunk per psum tile
NQC = S // QC        # 2
NSUB = QC // 512     # matmul sub-chunks per psum tile
HDH = HC * DH        # per-core Wo contraction = 512

F32 = mybir.dt.float32
F16 = mybir.dt.float16
EXP = mybir.ActivationFunctionType.Exp


def _emit(nc):
    xq = nc.dram_tensor("xq", [S, D], F32, kind="ExternalInput")
    xkv = nc.dram_tensor("xkv", [S, D], F32, kind="ExternalInput")
    wq = nc.dram_tensor("wq", [D, HDH], F32, kind="ExternalInput")
    wk = nc.dram_tensor("wk", [D, HDH], F32, kind="ExternalInput")
    wv = nc.dram_tensor("wv", [D, HDH], F32, kind="ExternalInput")
    bq = nc.dram_tensor("bq", [HDH], F32, kind="ExternalInput")
    bk = nc.dram_tensor("bk", [HDH], F32, kind="ExternalInput")
    bv = nc.dram_tensor("bv", [HDH], F32, kind="ExternalInput")
    wo = nc.dram_tensor("wo", [HDH, D], F32, kind="ExternalInput")
    out = nc.dram_tensor("out", [S, D], F32, kind="ExternalOutput")

    with tile.TileContext(nc) as tc:
        with tc.tile_pool(name="persist", bufs=1) as pers:
            # persistent SBUF tensors
            qt_sb = [pers.tile([P, S], F16, tag=f"qt{t}", name=f"qt{t}") for t in range(PAIRS)]
            kt_sb = [pers.tile([P, S], F16, tag=f"kt{t}", name=f"kt{t}") for t in range(PAIRS)]
            v_sb = [pers.tile([P, HDH], F16, tag=f"v{st}", name=f"v{st}") for st in range(NKT)]
            wo_sb = [pers.tile([P, D], F16, tag=f"wo{t}", name=f"wo{t}") for t in range(PAIRS)]
            ones_mm = pers.tile([1, 512], F16, tag="ones_mm")
            ones_red = pers.tile([P, 64], F16, tag="ones_red")
            bq_sb = pers.tile([1, HDH], F16, tag="bq")
            bk_sb = pers.tile([1, HDH], F16, tag="bk")
            bv_sb = pers.tile([1, HDH], F16, tag="bv")

            nc.vector.memset(ones_mm, 1.0)
            nc.vector.memset(ones_red, 1.0)
            nc.gpsimd.dma_start(out=bq_sb, in_=bq[None, :])
            nc.gpsimd.dma_start(out=bk_sb, in_=bk[None, :])
            nc.gpsimd.dma_start(out=bv_sb, in_=bv[None, :])

            # ---------------- Phase 1: projections ----------------
            with (
                tc.tile_pool(name="xt", bufs=9) as xt_pool,
                tc.tile_pool(name="stg", bufs=17) as stg_pool,
                tc.tile_pool(name="w", bufs=16) as w_pool,
                tc.tile_pool(name="pproj", bufs=4, space="PSUM") as pj,
            ):
                # Wo load (SWDGE cast-DMA fp32 -> fp16)
                for t in range(PAIRS):
                    nc.gpsimd.dma_start(out=wo_sb[t], in_=wo[t * P : (t + 1) * P, :])

                def load_xT(x_dram):
                    # Natural-layout SWDGE cast-DMA (fp32->fp16, contiguous
                    # rows at full HBM rate), then XBAR-transpose 128x128
                    # fp16 blocks into the [d, s] tiles the projections need.
                    nat = []
                    for st in range(NKT):
                        n_ = stg_pool.tile([P, D], F16, tag="stg")
                        nc.gpsimd.dma_start(
                            out=n_, in_=x_dram[st * P : (st + 1) * P, :]
                        )
                        nat.append(n_)
                    tiles = []
                    for d in range(DT):
                        t_ = xt_pool.tile([P, S], F16, tag="xt")
                        for st in range(NKT):
                            nc.sync.dma_start_transpose(
                                out=t_[:, st * P : (st + 1) * P],
                                in_=nat[st][:, d * P : (d + 1) * P],
                            )
                        tiles.append(t_)
                    return tiles

                def load_w(w_dram):
                    tiles = []
                    for d in range(DT):
                        t_ = w_pool.tile([P, HDH], F16, tag="w")
                        nc.gpsimd.dma_start(out=t_, in_=w_dram[d * P : (d + 1) * P, :])
                        tiles.append(t_)
                    return tiles

                def proj_T(x_tiles, w_tiles, b_sb, out_tiles):
                    # out_tiles[pair][128 = pair-dh, S] = W.T @ X.T + b
                    for t in range(PAIRS):
                        for c in range(S // 512):
                            ps = pj.tile([P, 512], F32, tag="pj")
                            for d in range(DT):
                                nc.tensor.matmul(
                                    ps,
                                    lhsT=w_tiles[d][:, t * P : (t + 1) * P],
                                    rhs=x_tiles[d][:, c * 512 : (c + 1) * 512],
                                    start=(d == 0),
                                    stop=False,
                                )
                            nc.tensor.matmul(
                                ps,
                                lhsT=b_sb[:, t * P : (t + 1) * P],
                                rhs=ones_mm,
                                start=False,
                                stop=True,
                            )
                            nc.vector.tensor_copy(
                                out=out_tiles[t][:, c * 512 : (c + 1) * 512], in_=ps
                            )

                xq_t = load_xT(xq)
                wq_t = load_w(wq)
                wk_t = load_w(wk)
                proj_T(xq_t, wq_t, bq_sb, qt_sb)

                xkv_t = load_xT(xkv)
                proj_T(xkv_t, wk_t, bk_sb, kt_sb)

                wv_t = load_w(wv)
                # V natural layout: [s-tile 128, (h dh) 512] = X @ Wv + bv
                for st in range(NKT):
                    ps = pj.tile([P, 512], F32, tag="pj")
                    for d in range(DT):
                        nc.tensor.matmul(
                            ps,
                            lhsT=xkv_t[d][:, st * P : (st + 1) * P],
                            rhs=wv_t[d],
                            start=(d == 0),
                            stop=False,
                        )
                    nc.tensor.matmul(
                        ps,
                        lhsT=ones_mm[:, :P],
                        rhs=bv_sb,
                        start=False,
                        stop=True,
                    )
                    nc.vector.tensor_copy(out=v_sb[st], in_=ps)

            # ---------------- Phase 2: attention + out projection ----------------
            with (
                tc.tile_pool(name="psc", bufs=2, space="PSUM") as psc,
                tc.tile_pool(name="pxt", bufs=1, space="PSUM") as pxt,
                tc.tile_pool(name="pout", bufs=2, space="PSUM") as pout,
                tc.tile_pool(name="et", bufs=4) as et_pool,
                tc.tile_pool(name="accp", bufs=4) as acc_pool,
                tc.tile_pool(name="rec", bufs=2) as rec_pool,
                tc.tile_pool(name="xtsb", bufs=8) as xtsb_pool,
            ):
                for qc in range(NQC):
                    xts = []
                    for pr in range(PAIRS):
                        h0, h1 = 2 * pr, 2 * pr + 1
                        acc0 = acc_pool.tile([P, QC], F16, tag="acc")
                        acc1 = acc_pool.tile([P, QC], F16, tag="acc")
                        xt_ps = pxt.tile([P, QC], F32, tag="xt")
                        for kt in range(NKT):
                            ps_a = psc.tile([P, QC], F32, tag="sc")
                            ps_b = psc.tile([P, QC], F32, tag="sc")
                            ksl = slice(kt * P, (kt + 1) * P)
                            for sub in range(NSUB):
                                sl = slice(sub * 512, (sub + 1) * 512)
                                qsl = slice(
                                    qc * QC + sub * 512, qc * QC + (sub + 1) * 512
                                )
                                # scoresT[k, q] for the two heads, row-packed
                                nc.tensor.matmul(
                                    ps_a[:, sl],
                                    lhsT=kt_sb[pr][0:64, ksl],
                                    rhs=qt_sb[pr][0:64, qsl],
                                    start=True,
                                    stop=True,
                                    tile_position=(0, 0),
                                )
                                nc.tensor.matmul(
                                    ps_b[:, sl],
                                    lhsT=kt_sb[pr][64:128, ksl],
                                    rhs=qt_sb[pr][64:128, qsl],
                                    start=True,
                                    stop=True,
                                    tile_position=(64, 0),
                                )
                            et0 = et_pool.tile([P, QC], F16, tag="et")
                            et1 = et_pool.tile([P, QC], F16, tag="et")
                            nc.scalar.activation(out=et0, in_=ps_a, func=EXP, scale=0.125)
                            nc.scalar.activation(out=et1, in_=ps_b, func=EXP, scale=0.125)
                            # partial softmax denominators (fp16, 16 terms per lane)
                            if kt == 0:
                                nc.vector.tensor_copy(out=acc0, in_=et0)
                                nc.vector.tensor_copy(out=acc1, in_=et1)
                            else:
                                nc.vector.tensor_add(out=acc0, in0=acc0, in1=et0)
                                nc.vector.tensor_add(out=acc1, in0=acc1, in1=et1)
                            # xT[dh, q] += V[k-tile].T-slice @ expT, heads col-packed
                            for sub in range(NSUB):
                                sl = slice(sub * 512, (sub + 1) * 512)
                                nc.tensor.matmul(
                                    xt_ps[0:64, sl],
                                    lhsT=v_sb[kt][:, h0 * DH : (h0 + 1) * DH],
                                    rhs=et0[:, sl],
                                    start=(kt == 0),
                                    stop=(kt == NKT - 1),
                                    tile_position=(0, 0),
                                    skip_group_check=True,
                                )
                                nc.tensor.matmul(
                                    xt_ps[64:128, sl],
                                    lhsT=v_sb[kt][:, h1 * DH : (h1 + 1) * DH],
                                    rhs=et1[:, sl],
                                    start=(kt == 0),
                                    stop=(kt == NKT - 1),
                                    tile_position=(0, 64),
                                    skip_group_check=True,
                                )
                        # denominators: partition-reduce + broadcast in one matmul
                        bs = psc.tile([P, QC], F32, tag="sc")
                        for sub in range(NSUB):
                            sl = slice(sub * 512, (sub + 1) * 512)
                            nc.tensor.matmul(
                                bs[0:64, sl],
                                lhsT=ones_red,
                                rhs=acc0[:, sl],
                                start=True,
                                stop=True,
                                tile_position=(0, 0),
                                skip_group_check=True,
                            )
                            nc.tensor.matmul(
                                bs[64:128, sl],
                                lhsT=ones_red,
                                rhs=acc1[:, sl],
                                start=True,
                                stop=True,
                                tile_position=(0, 64),
                                skip_group_check=True,
                            )
                        rec = rec_pool.tile([P, QC], F32, tag="rec")
                        nc.vector.reciprocal_approx_fast(out=rec, in_=bs)
                        xt_sb = xtsb_pool.tile([P, QC], F16, tag="xtsb")
                        nc.vector.tensor_mul(out=xt_sb, in0=xt_ps, in1=rec)
                        xts.append(xt_sb)
                    # out projection for this q chunk
                    for qt_ in range(QC // P):
                        for dc in range(D // 512):
                            po = pout.tile([P, 512], F32, tag="po")
                            for pr in range(PAIRS):
                                nc.tensor.matmul(
                                    po,
                                    lhsT=xts[pr][:, qt_ * P : (qt_ + 1) * P],
                                    rhs=wo_sb[pr][:, dc * 512 : (dc + 1) * 512],
                                    start=(pr == 0),
                                    stop=(pr == PAIRS - 1),
                                )
                            osb = xtsb_pool.tile([P, 512], F32, tag="osb")
                            nc.vector.tensor_copy(out=osb, in_=po)
                            q0 = qc * QC + qt_ * P
                            nc.gpsimd.dma_start(
                                out=out[q0 : q0 + P, dc * 512 : (dc + 1) * 512],
                                in_=osb,
                            )
    return nc


_NC_CACHE = None
LAST_RESULTS = None


def _get_nc():
    global _NC_CACHE
    if _NC_CACHE is None:
        nc = bacc.Bacc(None, target_bir_lowering=False)
        _emit(nc)
        nc.compile()
        _NC_CACHE = nc
    return _NC_CACHE


def kernel(**inputs):
    global LAST_RESULTS
    inputs_q = np.ascontiguousarray(inputs["inputs_q"], np.float32)
    inputs_kv = np.ascontiguousarray(inputs["inputs_kv"], np.float32)
    Wq = np.asarray(inputs["Wq"], np.float32)
    Wk = np.asarray(inputs["Wk"], np.float32)
    Wv = np.asarray(inputs["Wv"], np.float32)
    bq = np.asarray(inputs["bq"], np.float32)
    bk = np.asarray(inputs["bk"], np.float32)
    bv = np.asarray(inputs["bv"], np.float32)
    Wo = np.asarray(inputs["Wo"], np.float32)
    bo = np.asarray(inputs["bo"], np.float32)

    nc = _get_nc()

    in_maps = []
    for core in range(8):
        b, g = core // 2, core % 2
        hsl = slice(g * HC, (g + 1) * HC)
        in_maps.append(
            {
                "xq": inputs_q[b],
                "xkv": inputs_kv[b],
                "wq": np.ascontiguousarray(Wq[:, hsl, :].reshape(D, HDH)),
                "wk": np.ascontiguousarray(Wk[:, hsl, :].reshape(D, HDH)),
                "wv": np.ascontiguousarray(Wv[:, hsl, :].reshape(D, HDH)),
                "bq": np.ascontiguousarray(bq[hsl].reshape(HDH)),
                "bk": np.ascontiguousarray(bk[hsl].reshape(HDH)),
                "bv": np.ascontiguousarray(bv[hsl].reshape(HDH)),
                "wo": np.ascontiguousarray(Wo[hsl].reshape(HDH, D)),
            }
        )

    res = run_bass_kernel_spmd(
        nc,
        in_maps,
        core_ids=list(range(8)),
        trace=bool(int(os.environ.get("KERNEL_TRACE", "0"))),
    )
    LAST_RESULTS = res

    out = np.empty((B, S, D), np.float32)
    for b in range(B):
        out[b] = res.results[2 * b]["out"] + res.results[2 * b + 1]["out"] + bo
    return out


# revision 11
# speedup vs baseline: 5.3064x; 2.0031x over previous
"""Multi-head dot-product attention on 8 TRN2 NeuronCores.

Problem: B=4, S=2048, D=1024, H=16, DH=64 (fp32 reference).

Sharding: 8 shards = 4 batches x 2 head-halves. Each core computes, for one
batch b and 8 heads, the QKV projections, attention, and its partial output
projection. The host sums the two half-head partials per batch (the Wo
contraction all-reduce) and adds bo.

Per-core kernel layout (all matmul contraction dims on SBUF partitions):
  - XqT/XkvT: x loaded transposed, [D(128-tiles), S] fp32; projections run in
    float32r (full-rate PE) so no input casts are needed.
  - QT/KT: [128 = head-pair (2x64 dh), S] fp16 - produced directly transposed
    by using W as lhsT. Head pairs stacked so that the K=64 scores matmuls for
    the two heads row-pack onto the PE array (tile_position rows 0-63/64-127).
  - scoresT: [k-tile 128, q 1024] PSUM; exp on ACT (scale=1/8 folded in,
    no max-subtraction: scores ~ N(0,1), |s| < ~7, exp is safe in fp32/fp16).
  - softmax denominators: DVE accumulates expT k-tiles into an fp16 partial-sum
    tile; a ones[128,64] matmul reduces partitions AND broadcasts, giving
    per-head denominator rows aligned with xT; fast reciprocal on DVE.
  - PV: xT[dh, q] accumulated over k-tiles, two heads col-packed
    (tile_position cols 0-63/64-127) -> xT stacked [128, q] ready as lhsT for
    the Wo projection. Normalization fused into the PSUM->SBUF evacuation.
  - out projection: out[q,d] accumulated over 4 head-pairs, DMA'd to DRAM
    straight from PSUM.
"""

import os

import numpy as np

import concourse.bass as bass
from concourse import bacc
import concourse.mybir as mybir
import concourse.tile as tile
from concourse.bass_utils import run_bass_kernel_spmd

B, S, D, H, DH = 4, 2048, 1024, 16, 64
P = 128
HC = H // 2          # heads per core = 8
PAIRS = HC // 2      # head pairs per core = 4
DT = D // P          # projection contraction tiles = 8
NKT = S // P         # key tiles = 16
QC = 1024            # q chunk per psum tile
NQC = S // QC        # 2
NSUB = QC // 512     # matmul sub-chunks per psum tile
HDH = HC * DH        # per-core Wo contraction = 512

F32 = mybir.dt.float32
F16 = mybir.dt.float16
EXP = mybir.ActivationFunctionType.Exp


def _emit(nc):
    xq = nc.dram_tensor("xq", [S, D], F16, kind="ExternalInput")
    xkv = nc.dram_tensor("xkv", [S, D], F16, kind="ExternalInput")
    wq = nc.dram_tensor("wq", [D, HDH], F16, kind="ExternalInput")
    wk = nc.dram_tensor("wk", [D, HDH], F16, kind="ExternalInput")
    wv = nc.dram_tensor("wv", [D, HDH], F16, kind="ExternalInput")
    bq = nc.dram_tensor("bq", [HDH], F16, kind="ExternalInput")
    bk = nc.dram_tensor("bk", [HDH], F16, kind="ExternalInput")
    bv = nc.dram_tensor("bv", [HDH], F16, kind="ExternalInput")
    wo = nc.dram_tensor("wo", [HDH, D], F16, kind="ExternalInput")
    out = nc.dram_tensor("out", [S, D], F32, kind="ExternalOutput")

    with tile.TileContext(nc) as tc:
        with tc.tile_pool(name="persist", bufs=1) as pers:
            # persistent SBUF tensors
            qt_sb = [pers.tile([P, S], F16, tag=f"qt{t}", name=f"qt{t}") for t in range(PAIRS)]
            kt_sb = [pers.tile([P, S], F16, tag=f"kt{t}", name=f"kt{t}") for t in range(PAIRS)]
            v_sb = [pers.tile([P, HDH], F16, tag=f"v{st}", name=f"v{st}") for st in range(NKT)]
            wo_sb = [pers.tile([P, D], F16, tag=f"wo{t}", name=f"wo{t}") for t in range(PAIRS)]
            ones_mm = pers.tile([1, 512], F16, tag="ones_mm")
            ones_red = pers.tile([P, 64], F16, tag="ones_red")
            bq_sb = pers.tile([1, HDH], F16, tag="bq")
            bk_sb = pers.tile([1, HDH], F16, tag="bk")
            bv_sb = pers.tile([1, HDH], F16, tag="bv")

            nc.vector.memset(ones_mm, 1.0)
            nc.vector.memset(ones_red, 1.0)
            nc.sync.dma_start(out=bq_sb, in_=bq[None, :])
            nc.sync.dma_start(out=bk_sb, in_=bk[None, :])
            nc.sync.dma_start(out=bv_sb, in_=bv[None, :])

            # ---------------- Phase 1: projections ----------------
            with (
                tc.tile_pool(name="xt", bufs=9) as xt_pool,
                tc.tile_pool(name="w", bufs=16) as w_pool,
                tc.tile_pool(name="pproj", bufs=4, space="PSUM") as pj,
            ):
                # Wo load
                for t in range(PAIRS):
                    nc.sync.dma_start(out=wo_sb[t], in_=wo[t * P : (t + 1) * P, :])

                def load_xT(x_dram):
                    # One big M2S XBAR transpose per d-tile (fp16, DRAM->SBUF)
                    tiles = []
                    for d in range(DT):
                        t_ = xt_pool.tile([P, S], F16, tag="xt")
                        nc.sync.dma_start_transpose(
                            out=t_, in_=x_dram[:, d * P : (d + 1) * P]
                        )
                        tiles.append(t_)
                    return tiles

                def load_w(w_dram):
                    tiles = []
                    for d in range(DT):
                        t_ = w_pool.tile([P, HDH], F16, tag="w")
                        nc.sync.dma_start(out=t_, in_=w_dram[d * P : (d + 1) * P, :])
                        tiles.append(t_)
                    return tiles

                def proj_T(x_tiles, w_tiles, b_sb, out_tiles):
                    # out_tiles[pair][128 = pair-dh, S] = W.T @ X.T + b
                    for t in range(PAIRS):
                        for c in range(S // 512):
                            ps = pj.tile([P, 512], F32, tag="pj")
                            for d in range(DT):
                                nc.tensor.matmul(
                                    ps,
                                    lhsT=w_tiles[d][:, t * P : (t + 1) * P],
                                    rhs=x_tiles[d][:, c * 512 : (c + 1) * 512],
                                    start=(d == 0),
                                    stop=False,
                                )
                            nc.tensor.matmul(
                                ps,
                                lhsT=b_sb[:, t * P : (t + 1) * P],
                                rhs=ones_mm,
                                start=False,
                                stop=True,
                            )
                            nc.vector.tensor_copy(
                                out=out_tiles[t][:, c * 512 : (c + 1) * 512], in_=ps
                            )

                xq_t = load_xT(xq)
                wq_t = load_w(wq)
                wk_t = load_w(wk)
                proj_T(xq_t, wq_t, bq_sb, qt_sb)

                xkv_t = load_xT(xkv)
                proj_T(xkv_t, wk_t, bk_sb, kt_sb)

                wv_t = load_w(wv)
                # V natural layout: [s-tile 128, (h dh) 512] = X @ Wv + bv
                for st in range(NKT):
                    ps = pj.tile([P, 512], F32, tag="pj")
                    for d in range(DT):
                        nc.tensor.matmul(
                            ps,
                            lhsT=xkv_t[d][:, st * P : (st + 1) * P],
                            rhs=wv_t[d],
                            start=(d == 0),
                            stop=False,
                        )
                    nc.tensor.matmul(
                        ps,
                        lhsT=ones_mm[:, :P],
                        rhs=bv_sb,
                        start=False,
                        stop=True,
                    )
                    nc.vector.tensor_copy(out=v_sb[st], in_=ps)

            # ---------------- Phase 2: attention + out projection ----------------
            with (
                tc.tile_pool(name="psc", bufs=2, space="PSUM") as psc,
                tc.tile_pool(name="pxta", bufs=1, space="PSUM") as pxta,
                tc.tile_pool(name="pxtb", bufs=1, space="PSUM") as pxtb,
                tc.tile_pool(name="et", bufs=4) as et_pool,
                tc.tile_pool(name="accp", bufs=4) as acc_pool,
                tc.tile_pool(name="rec", bufs=2) as rec_pool,
                tc.tile_pool(name="xtsb", bufs=8) as xtsb_pool,
            ):
                for qc in range(NQC):
                    xts = []
                    for pr in range(PAIRS):
                        h0, h1 = 2 * pr, 2 * pr + 1
                        acc0 = acc_pool.tile([P, QC], F16, tag="acc")
                        acc1 = acc_pool.tile([P, QC], F16, tag="acc")
                        xt_a = pxta.tile([P, QC], F32, tag="xta")
                        xt_b = pxtb.tile([P, QC], F32, tag="xtb")
                        for kt in range(NKT):
                            ps_a = psc.tile([P, QC], F32, tag="sc")
                            ps_b = psc.tile([P, QC], F32, tag="sc")
                            ksl = slice(kt * P, (kt + 1) * P)
                            for sub in range(NSUB):
                                sl = slice(sub * 512, (sub + 1) * 512)
                                qsl = slice(
                                    qc * QC + sub * 512, qc * QC + (sub + 1) * 512
                                )
                                # scoresT[k, q] for the two heads, row-packed
                                nc.tensor.matmul(
                                    ps_a[:, sl],
                                    lhsT=kt_sb[pr][0:64, ksl],
                                    rhs=qt_sb[pr][0:64, qsl],
                                    start=True,
                                    stop=True,
                                    tile_position=(0, 0),
                                )
                                nc.tensor.matmul(
                                    ps_b[:, sl],
                                    lhsT=kt_sb[pr][64:128, ksl],
                                    rhs=qt_sb[pr][64:128, qsl],
                                    start=True,
                                    stop=True,
                                    tile_position=(64, 0),
                                )
                            et0 = et_pool.tile([P, QC], F16, tag="et")
                            et1 = et_pool.tile([P, QC], F16, tag="et")
                            nc.scalar.activation(out=et0, in_=ps_a, func=EXP, scale=0.125)
                            nc.scalar.activation(out=et1, in_=ps_b, func=EXP, scale=0.125)
                            # partial softmax denominators (fp16, 16 terms per lane)
                            if kt == 0:
                                nc.vector.tensor_copy(out=acc0, in_=et0)
                                nc.vector.tensor_copy(out=acc1, in_=et1)
                            else:
                                nc.vector.tensor_add(out=acc0, in0=acc0, in1=et0)
                                nc.vector.tensor_add(out=acc1, in0=acc1, in1=et1)
                            # xT[dh, q] += V[k-tile].T-slice @ expT, heads col-packed
                            for sub in range(NSUB):
                                sl = slice(sub * 512, (sub + 1) * 512)
                                nc.tensor.matmul(
                                    xt_a[0:64, sl],
                                    lhsT=v_sb[kt][:, h0 * DH : (h0 + 1) * DH],
                                    rhs=et0[:, sl],
                                    start=(kt == 0),
                                    stop=(kt == NKT - 1),
                                    tile_position=(0, 0),
                                    skip_group_check=True,
                                )
                                nc.tensor.matmul(
                                    xt_b[64:128, sl],
                                    lhsT=v_sb[kt][:, h1 * DH : (h1 + 1) * DH],
                                    rhs=et1[:, sl],
                                    start=(kt == 0),
                                    stop=(kt == NKT - 1),
                                    tile_position=(0, 64),
                                    skip_group_check=True,
                                )
                        # denominators: partition-reduce + broadcast in one matmul
                        bs = psc.tile([P, QC], F32, tag="sc")
                        for sub in range(NSUB):
                            sl = slice(sub * 512, (sub + 1) * 512)
                            nc.tensor.matmul(
                                bs[0:64, sl],
                                lhsT=ones_red,
                                rhs=acc0[:, sl],
                                start=True,
                                stop=True,
                                tile_position=(0, 0),
                                skip_group_check=True,
                            )
                            nc.tensor.matmul(
                                bs[64:128, sl],
                                lhsT=ones_red,
                                rhs=acc1[:, sl],
                                start=True,
                                stop=True,
                                tile_position=(0, 64),
                                skip_group_check=True,
                            )
                        rec = rec_pool.tile([P, QC], F32, tag="rec")
                        nc.vector.reciprocal_approx_fast(out=rec, in_=bs)
                        xt_sb = xtsb_pool.tile([P, QC], F16, tag="xtsb")
                        nc.vector.tensor_mul(
                            out=xt_sb[0:64, :], in0=xt_a[0:64, :], in1=rec[0:64, :]
                        )
                        nc.vector.tensor_mul(
                            out=xt_sb[64:128, :], in0=xt_b[64:128, :], in1=rec[64:128, :]
                        )
                        xts.append(xt_sb)
                    # out projection for this q chunk
                    for qt_ in range(QC // P):
                        for dc in range(D // 512):
                            po = psc.tile([P, 512], F32, tag="sc")
                            for pr in range(PAIRS):
                                nc.tensor.matmul(
                                    po,
                                    lhsT=xts[pr][:, qt_ * P : (qt_ + 1) * P],
                                    rhs=wo_sb[pr][:, dc * 512 : (dc + 1) * 512],
                                    start=(pr == 0),
                                    stop=(pr == PAIRS - 1),
                                )
                            osb = xtsb_pool.tile([P, 512], F32, tag="osb")
                            nc.vector.tensor_copy(out=osb, in_=po)
                            q0 = qc * QC + qt_ * P
                            nc.gpsimd.dma_start(
                                out=out[q0 : q0 + P, dc * 512 : (dc + 1) * 512],
                                in_=osb,
                            )
    return nc


_NC_CACHE = None
LAST_RESULTS = None


def _get_nc():
    global _NC_CACHE
    if _NC_CACHE is None:
        nc = bacc.Bacc(None, target_bir_lowering=False)
        _emit(nc)
        nc.compile()
        _NC_CACHE = nc
    return _NC_CACHE


def kernel(**inputs):
    global LAST_RESULTS
    inputs_q = np.ascontiguousarray(inputs["inputs_q"], np.float16)
    inputs_kv = np.ascontiguousarray(inputs["inputs_kv"], np.float16)
    Wq = np.asarray(inputs["Wq"], np.float16)
    Wk = np.asarray(inputs["Wk"], np.float16)
    Wv = np.asarray(inputs["Wv"], np.float16)
    bq = np.asarray(inputs["bq"], np.float16)
    bk = np.asarray(inputs["bk"], np.float16)
    bv = np.asarray(inputs["bv"], np.float16)
    Wo = np.asarray(inputs["Wo"], np.float16)
    bo = np.asarray(inputs["bo"], np.float32)

    nc = _get_nc()

    in_maps = []
    for core in range(8):
        b, g = core // 2, core % 2
        hsl = slice(g * HC, (g + 1) * HC)
        in_maps.append(
            {
                "xq": inputs_q[b],
                "xkv": inputs_kv[b],
                "wq": np.ascontiguousarray(Wq[:, hsl, :].reshape(D, HDH)),
                "wk": np.ascontiguousarray(Wk[:, hsl, :].reshape(D, HDH)),
                "wv": np.ascontiguousarray(Wv[:, hsl, :].reshape(D, HDH)),
                "bq": np.ascontiguousarray(bq[hsl].reshape(HDH)),
                "bk": np.ascontiguousarray(bk[hsl].reshape(HDH)),
                "bv": np.ascontiguousarray(bv[hsl].reshape(HDH)),
                "wo": np.ascontiguousarray(Wo[hsl].reshape(HDH, D)),
            }
        )

    res = run_bass_kernel_spmd(
        nc,
        in_maps,
        core_ids=list(range(8)),
        trace=bool(int(os.environ.get("KERNEL_TRACE", "0"))),
    )
    LAST_RESULTS = res

    out = np.empty((B, S, D), np.float32)
    for b in range(B):
        out[b] = res.results[2 * b]["out"] + res.results[2 * b + 1]["out"] + bo
    return out
